# revision 43
# baseline (speedup 1.0000x reference)
"""AxialAttention Trainium2 kernel (8 NeuronCores, SPMD).

Sharding: core = b*4 + q  (b in {0,1}, q in {0..3}); each core handles one
batch element and a 10-row H-slab, with all 256 channels. The three
"branches" in the reference are numerically identical (h=w=d=40 and the
reshape ignores the axis names), so out = 3 * branch; the factor 3 is folded
into wp. The attention scale is folded into wq/bq.

Layouts use a "double deck": channel c (<128) lives at partition band 0-39,
channel c+128 at band 64-103 (both legal matmul base partitions). Pivot
transposes place the two decks in one psum tile via tile_position col
offsets, so psum->SBUF evacuations move both decks per op.

The slice loop is software-pipelined for the in-order PE queue:
  phase 1: pivot transposes(i) interleaved with pivot-back+wp(i-1)
  phase 2: attention(i) (scores issued one group ahead of AV) interleaved
           with the qkv conv of slice i+1.
"""

import sys

sys.path.insert(0, "/opt/trn_rl_repo")

import numpy as np
import ml_dtypes
from contextlib import ExitStack

import concourse.bass as bass
import concourse.tile as tile
from concourse import bacc, mybir
from concourse.bass_utils import run_bass_kernel_spmd
from concourse.masks import make_identity

BF16 = mybir.dt.bfloat16
F32 = mybir.dt.float32

B, C, H, W, D = 2, 256, 40, 40, 40
HEADS = 8
HD = C // HEADS
SCALE = HD ** -0.5
N_CORES = 8
SLAB = H // 4          # 10 H-rows per core
WD = W * D             # 1600
NSLAB = SLAB * WD      # 16000
CHALF = 128


def _merge(a, b):
    """Proportionally interleave two chunk lists, preserving each order."""
    out = []
    na, nb = len(a), len(b)
    ia = ib = 0
    while ia < na or ib < nb:
        if ib >= nb or (ia * (nb + 1) <= ib * (na + 1) and ia < na):
            out.append(a[ia])
            ia += 1
        else:
            out.append(b[ib])
            ib += 1
    return out


def _build_nc():
    nc = bacc.Bacc(
        "TRN2",
        target_bir_lowering=False,
        debug=False,
        num_devices=N_CORES,
    )
    x_d = nc.declare_dram_parameter("x", [C, NSLAB], BF16, isOutput=False)
    wqkv_d = nc.declare_dram_parameter("wqkv", [C, 3 * C], BF16, isOutput=False)
    bqkv_d = nc.declare_dram_parameter("bqkv", [3 * C, 1], F32, isOutput=False)
    wp_d = nc.declare_dram_parameter("wp3", [C, C], BF16, isOutput=False)
    bp_d = nc.declare_dram_parameter("bp", [C, 1], F32, isOutput=False)
    out_d = nc.declare_dram_parameter("out", [C, NSLAB], F32, isOutput=True)

    IDENT = mybir.ActivationFunctionType.Identity
    EXP = mybir.ActivationFunctionType.Exp
    MULT = mybir.AluOpType.mult

    with ExitStack() as ctx:
        tc = ctx.enter_context(tile.TileContext(nc))
        const = ctx.enter_context(tc.tile_pool(name="const", bufs=1))
        xp = ctx.enter_context(tc.tile_pool(name="xp", bufs=2))
        qkvp = ctx.enter_context(tc.tile_pool(name="qkvp", bufs=2))
        attp = ctx.enter_context(tc.tile_pool(name="attp", bufs=2))
        ep = ctx.enter_context(tc.tile_pool(name="ep", bufs=3))
        oallp = ctx.enter_context(tc.tile_pool(name="oallp", bufs=2))
        # branch/bt are produced and consumed within one pbwp (a full slice
        # before their next writer): single buffer suffices
        brp = ctx.enter_context(tc.tile_pool(name="brp", bufs=1))
        btp = ctx.enter_context(tc.tile_pool(name="btp", bufs=1))
        recp = ctx.enter_context(tc.tile_pool(name="recp", bufs=4))
        outp = ctx.enter_context(tc.tile_pool(name="outp", bufs=2))
        # conv and wp psums share one 4-deep tag; scores shares ps_t with the
        # pivot transposes (time-disjoint phases); o_ps and pb share ps_x
        ps_cs = ctx.enter_context(tc.tile_pool(name="ps_cs", bufs=4, space="PSUM"))
        ps_t = ctx.enter_context(tc.tile_pool(name="ps_t", bufs=2, space="PSUM"))
        ps_x = ctx.enter_context(tc.tile_pool(name="ps_x", bufs=2, space="PSUM"))

        ident = const.tile([128, 128], BF16)
        make_identity(nc, ident[:])

        # qkv weights DMA goes first: it gates the very first conv matmul.
        # Split by contraction half — the k=0 matmuls only need the first.
        # wp/bp (needed ~12us in) are issued after x(0) — see below.
        wqkv_sb = const.tile([128, 2, 3 * C], BF16)
        wqkv_src = wqkv_d.ap().rearrange("(ko ki) m -> ki ko m", ki=128)
        nc.sync.dma_start(wqkv_sb[:, 0], wqkv_src[:, 0])
        nc.sync.dma_start(wqkv_sb[:, 1], wqkv_src[:, 1])
        bqkv_sb = const.tile([128, 6, 1], F32)
        nc.sync.dma_start(
            bqkv_sb[:], bqkv_d.ap().rearrange("(mo mi) one -> mi mo one", mi=128)
        )
        wp_sb = const.tile([128, 2, C], BF16)
        bp_sb = const.tile([128, 2, 1], F32)

        def load_weights_tail():
            nc.sync.dma_start(
                wp_sb[:], wp_d.ap().rearrange("(ko ki) m -> ki ko m", ki=128)
            )
            nc.sync.dma_start(
                bp_sb[:], bp_d.ap().rearrange("(mo mi) one -> mi mo one", mi=128)
            )

        # channel-pair groups (deck covers c_local and c_local+128)
        groups = []
        c0 = 0
        while c0 < CHALF:
            groups.append((c0, min(12, CHALF - c0)))
            c0 += 12

        def load_x(i, defer=False):
            # split into 4 chunks: the first conv matmul only needs chunk 0,
            # and small chunks interleave with out-store DMAs on the shared
            # DMA engines instead of queuing behind a full-slice transfer.
            # With defer=True the dma_starts are returned as chunks so they
            # can be issued after the pivot-back xbar transposes.
            x_sb = xp.tile([128, 2, WD], BF16, name="x_sb")
            chunks = []
            for n in range(4):
                def ch(n=n):
                    nc.sync.dma_start(
                        x_sb[:, :, n * 400 : (n + 1) * 400],
                        x_d.ap()[
                            :, i * WD + n * 400 : i * WD + (n + 1) * 400
                        ].rearrange("(ko ki) n -> ki ko n", ki=128),
                    )
                chunks.append(ch)
            if not defer:
                for ch in chunks:
                    ch()
                chunks = []
            return x_sb, chunks

        def conv_emit(x_sb):
            # deck-interleaved padded layout (3 tensors q,k,v):
            #   q,k: col = w*128 + deck*64 + d      (t = 0, 1)
            #   v:   col = d*128 + deck*64 + w      (t = 2)
            # so one contiguous 128-col slice holds both decks of one w (or d)
            # and a single PE transpose pivots both. Pad cols 40-63 per deck
            # stay garbage; their transposed rows are never read.
            qkv_sb = qkvp.tile([128, 3, W * 128], BF16, name="qkv_sb")
            qk_view = qkv_sb.rearrange("p t (w g d) -> p t w g d", g=2, d=64)
            v_view = qkv_sb.rearrange("p t (d g w) -> p t d g w", g=2, w=64)
            chunks = []
            for m in range(6):
                for n in range(4):
                    def ch(m=m, n=n, x_sb=x_sb):
                        ps = ps_cs.tile(
                            [128, 512], F32, tag="ps_cs", name="conv_ps"
                        )[:, 0:400]
                        for k in range(2):
                            nc.tensor.matmul(
                                ps[:],
                                lhsT=wqkv_sb[:, k, m * 128 : (m + 1) * 128],
                                rhs=x_sb[:, k, n * 400 : (n + 1) * 400],
                                start=(k == 0),
                                stop=(k == 1),
                            )
                        t, g = m // 2, m % 2
                        if m < 4:
                            # psum cols enumerate (w outer, d inner)
                            dst = qk_view[:, t, n * 10 : (n + 1) * 10, g, 0:40]
                        else:
                            dst = v_view[:, t, 0:40, g, n * 10 : (n + 1) * 10].rearrange(
                                "p d w -> p w d"
                            )
                        # conv evacuation overlaps the attention phase, where
                        # the scalar engine also runs exp: put 1/3 on vector
                        if n == 0 or (n == 1 and m < 2):
                            nc.vector.tensor_scalar_add(dst, ps[:], bqkv_sb[:, m])
                        else:
                            nc.scalar.activation(
                                out=dst,
                                in_=ps[:],
                                func=IDENT,
                                bias=bqkv_sb[:, m],
                                scale=1.0,
                            )
                    chunks.append(ch)
            return qkv_sb, chunks

        def pivots_emit(qkv_sb):
            q_att = attp.tile([128, W * CHALF], BF16, tag="q_att", name="q_att")
            k_att = attp.tile([128, W * CHALF], BF16, tag="k_att", name="k_att")
            v_att = attp.tile([128, 41 * CHALF], BF16, tag="v_att", name="v_att")
            chunks = [
                lambda: nc.vector.memset(v_att[:, 40 * CHALF : 41 * CHALF], 1.0)
            ]
            # one transpose per (tensor, w-or-d): in [128ch, 128 cols] holds
            # both decks of one w (d-padded) -> out [128, 128] with deck0
            # rows 0-39 and deck1 rows 64-103 (pad rows 40-63/104-127 are
            # junk, never read).
            # q/k pivots first so the first scores' dependencies clear while
            # the v pivots (needed one stage later, at the first AV) still run
            # the scalar engine is idle during the pivot phase (exp/conv
            # evacs overlap attention): give it q's evacuations; the vector
            # engine (2x on bf16 psum) takes k and v
            for src, dst, eng in (
                (0, q_att, "scalar"),
                (1, k_att, "vector"),
                (2, v_att, "vector"),
            ):
                for wg in range(10):
                    def ch(wg=wg, src=src, dst=dst, eng=eng):
                        pst = ps_t.tile([128, 512], BF16, tag="pst", name="pst")
                        for wl in range(4):
                            w = wg * 4 + wl
                            nc.tensor.transpose(
                                pst[:, wl * 128 : (wl + 1) * 128],
                                qkv_sb[:, src, w * 128 : (w + 1) * 128],
                                ident[:],
                            )
                        if eng == "scalar":
                            nc.scalar.copy(
                                dst[0:104, wg * 512 : (wg + 1) * 512], pst[0:104, :]
                            )
                        else:
                            nc.vector.tensor_copy(
                                out=dst[0:104, wg * 512 : (wg + 1) * 512],
                                in_=pst[0:104, :],
                            )
                    chunks.append(ch)
            return (q_att, k_att, v_att), chunks

        def attn_emit(att):
            q_att, k_att, v_att = att
            k_v = k_att.rearrange("p (w c) -> p c w", c=CHALF)
            q_v = q_att.rearrange("p (w c) -> p c w", c=CHALF)
            vv = v_att.rearrange("p (d c) -> p c d", c=CHALF)
            o_all = oallp.tile([128, CHALF * W], BF16, name="o_all")

            def scores_stage(c0, gn):
                s_ps = ps_t.tile([128, 512], F32, tag="pst", name="s_ps")
                for j in range(gn):
                    for cc in range(2):
                        r0 = cc * 64
                        nc.tensor.matmul(
                            s_ps[r0 : r0 + 40, j * 40 : (j + 1) * 40],
                            lhsT=k_v[r0 : r0 + 40, c0 + j],
                            rhs=q_v[r0 : r0 + 40, c0 + j],
                            start=True,
                            stop=True,
                        )
                e_sb = ep.tile([128, 480], BF16, tag="e_sb", name="e_sb")
                nc.scalar.activation(
                    out=e_sb[0:104, : gn * 40], in_=s_ps[0:104, : gn * 40], func=EXP
                )
                return e_sb

            def av_stage(c0, gn, e_sb):
                o_ps = ps_x.tile([128, 512], F32, tag="ps_x", name="o_ps")
                for j in range(gn):
                    for cc in range(2):
                        r0 = cc * 64
                        nc.tensor.matmul(
                            o_ps[r0 : r0 + 41, j * 40 : (j + 1) * 40],
                            lhsT=vv[r0 : r0 + 40, c0 + j],
                            rhs=e_sb[r0 : r0 + 40, j * 40 : (j + 1) * 40],
                            start=True,
                            stop=True,
                        )
                nc.vector.tensor_copy(
                    out=o_all[0:105, c0 * 40 : c0 * 40 + gn * 40],
                    in_=o_ps[0:105, : gn * 40],
                )

            pend = [None]
            chunks = []
            for c0, gn in groups:
                def ch(c0=c0, gn=gn):
                    e_sb = scores_stage(c0, gn)
                    if pend[0] is not None:
                        av_stage(*pend[0])
                    pend[0] = (c0, gn, e_sb)
                chunks.append(ch)
            chunks.append(lambda: av_stage(*pend[0]))
            return o_all, chunks

        def pbwp_emit(o_all, i, last=False):
            branch_sb = brp.tile([128, 2, WD], BF16, name="branch_sb")
            out_sb = outp.tile([128, 2, WD], F32, name="out_sb")
            # pivot-back via DMA xbar transpose instead of PE: o_all is
            # [112 rows (d decks + denoms + junk), c*40+w cols] bf16 in SBUF;
            # one transpose per channel-half yields bt[c, w, q] = o_all[q,
            # c*40+w]. The c-split lets half 0 start after the 6th o_all
            # evacuation, mid-attention.
            chunks = []
            if not last:
                bt = btp.tile([128, W, 112], BF16, name="bt")
                rec = recp.tile([128, W, 2], F32, tag="rec", name="rec")
                for h in range(2):
                    def ch(h=h):
                        p0, p1 = h * 64, (h + 1) * 64
                        nc.sync.dma_start(
                            bt[p0:p1],
                            o_all[0:112, h * 2560 : (h + 1) * 2560],
                            transpose=True,
                        )
                        nc.vector.reciprocal(rec[p0:p1, :, 0], bt[p0:p1, :, 40])
                        nc.vector.reciprocal(rec[p0:p1, :, 1], bt[p0:p1, :, 104])
                    chunks.append(ch)
                # full-width multiplies, one per wp n-chunk's w-range
                for q in range(4):
                    def ch(q=q):
                        w0, w1 = q * 10, (q + 1) * 10
                        for cc in range(2):
                            nc.vector.tensor_tensor(
                                branch_sb[:, cc].rearrange(
                                    "p (w d) -> p w d", d=40
                                )[:, w0:w1],
                                bt[:, w0:w1, cc * 64 : cc * 64 + 40],
                                rec[:, w0:w1, cc : cc + 1].to_broadcast(
                                    (128, 10, 40)
                                ),
                                MULT,
                            )
                    chunks.append(ch)
            else:
                # final slice: PE pivot-back — its latency chain is shorter
                # than the xbar round-trip, and nothing overlaps the tail
                o_v = o_all.rearrange("p (c w) -> p w c", w=W)
                for wb in range(5):
                    def ch(wb=wb):
                        pb_full = ps_x.tile(
                            [128, 1024], BF16, tag="ps_x", name="pb"
                        )
                        pb = pb_full[:, 0:848]
                        for wl in range(8):
                            w = wb * 8 + wl
                            nc.tensor.transpose(
                                pb[:, wl * 106 : wl * 106 + 105],
                                o_v[0:105, w, :],
                                ident[0:105, 0:105],
                            )
                        pb_v = pb.rearrange("p (w q) -> p w q", q=106)
                        rec = recp.tile([128, 8, 2], F32, tag="rec", name="rec")
                        nc.vector.reciprocal(rec[:, :, 0], pb_v[:, :, 40])
                        nc.vector.reciprocal(rec[:, :, 1], pb_v[:, :, 104])
                        for cc in range(2):
                            nc.vector.tensor_tensor(
                                branch_sb[:, cc].rearrange(
                                    "p (w d) -> p w d", d=40
                                )[:, wb * 8 : wb * 8 + 8],
                                pb_v[:, :, cc * 64 : cc * 64 + 40],
                                rec[:, :, cc : cc + 1].to_broadcast((128, 8, 40)),
                                MULT,
                            )
                    chunks.append(ch)
            # n-major so each store chunk can fire right after its two wp
            # evacuations; on the last slice both engines are idle, so
            # alternate the evacuations between them to shorten the tail
            for n in range(4):
                for m in range(2):
                    def ch(m=m, n=n):
                        ps = ps_cs.tile(
                            [128, 512], F32, tag="ps_cs", name="wp_ps"
                        )[:, 0:400]
                        for k in range(2):
                            nc.tensor.matmul(
                                ps[:],
                                lhsT=wp_sb[:, k, m * 128 : (m + 1) * 128],
                                rhs=branch_sb[:, k, n * 400 : (n + 1) * 400],
                                start=(k == 0),
                                stop=(k == 1),
                            )
                        if last and m == 1:
                            nc.vector.tensor_scalar_add(
                                out_sb[:, m, n * 400 : (n + 1) * 400],
                                ps[:],
                                bp_sb[:, m],
                            )
                        else:
                            nc.scalar.activation(
                                out=out_sb[:, m, n * 400 : (n + 1) * 400],
                                in_=ps[:],
                                func=IDENT,
                                bias=bp_sb[:, m],
                                scale=1.0,
                            )
                    chunks.append(ch)

                # split store: chunk dn only needs wp evacs (0,dn) and (1,dn)
                def dma_ch(dn=n):
                    nc.scalar.dma_start(
                        out_d.ap()[
                            :, i * WD + dn * 400 : i * WD + (dn + 1) * 400
                        ].rearrange("(ko ki) n -> ki ko n", ki=128),
                        out_sb[:, :, dn * 400 : (dn + 1) * 400],
                    )
                chunks.append(dma_ch)
            return chunks

        # ---- software-pipelined slice loop
        x_cur, _ = load_x(0)
        load_weights_tail()
        qkv_cur, conv_ch = conv_emit(x_cur)
        for ch in conv_ch:
            ch()
        pending_pbwp = []
        for i in range(SLAB):
            if i + 1 < SLAB:
                # defer the x dma_starts to after the pivot phase so the
                # pivot-back xbar transposes lead the DMA-engine queue
                x_nxt, x_ch = load_x(i + 1, defer=True)
            else:
                x_ch = []
            att, piv_ch = pivots_emit(qkv_cur)
            # a few pivot chunks lead the merge: the pivot-back transposes in
            # pending_pbwp wait on the previous slice's last o_all
            # evacuation, so give that copy time to land
            for ch in piv_ch[:4] + _merge(piv_ch[4:], pending_pbwp):
                ch()
            o_all, attn_ch = attn_emit(att)
            if i + 1 < SLAB:
                qkv_nxt, conv_ch = conv_emit(x_nxt)
            else:
                qkv_nxt, conv_ch = None, []
            for ch in x_ch + _merge(attn_ch, conv_ch):
                ch()
            pending_pbwp = pbwp_emit(o_all, i, last=(i == SLAB - 1))
            qkv_cur = qkv_nxt
        for ch in pending_pbwp:
            ch()

    nc.compile()
    return nc


_NC_CACHE = None


def _get_nc():
    global _NC_CACHE
    if _NC_CACHE is None:
        _NC_CACHE = _build_nc()
    return _NC_CACHE


def make_in_maps(x, wq, bq, wk, bk, wv, bv, wp, bp):
    bf = ml_dtypes.bfloat16
    wqkv = np.concatenate(
        [wq.T * SCALE, wk.T, wv.T], axis=1
    ).astype(bf)  # [C, 3C], lhsT layout (c_in rows, c_out cols)
    bqkv = np.concatenate([bq * SCALE, bk, bv]).reshape(3 * C, 1).astype(np.float32)
    wp3 = (3.0 * wp).T.astype(bf)  # [C, C]
    bp_ = bp.reshape(C, 1).astype(np.float32)
    in_maps = []
    for core in range(N_CORES):
        b = core // 4
        r0 = (core % 4) * SLAB
        x_slab = np.ascontiguousarray(
            x[b, :, r0 : r0 + SLAB].reshape(C, NSLAB)
        ).astype(bf)
        in_maps.append(
            {"x": x_slab, "wqkv": wqkv, "bqkv": bqkv, "wp3": wp3, "bp": bp_}
        )
    return in_maps


def run_on_cores(in_maps, **kw):
    nc = _get_nc()
    return run_bass_kernel_spmd(nc, in_maps, core_ids=list(range(N_CORES)), **kw)


def kernel(x, wq, bq, wk, bk, wv, bv, wp, bp):
    x = np.asarray(x, dtype=np.float32)
    in_maps = make_in_maps(
        x,
        np.asarray(wq, np.float32),
        np.asarray(bq, np.float32),
        np.asarray(wk, np.float32),
        np.asarray(bk, np.float32),
        np.asarray(wv, np.float32),
        np.asarray(bv, np.float32),
        np.asarray(wp, np.float32),
        np.asarray(bp, np.float32),
    )
    res = run_on_cores(in_maps)
    out = np.empty((B, C, H, W, D), np.float32)
    for core in range(N_CORES):
        b = core // 4
        r0 = (core % 4) * SLAB
        out[b, :, r0 : r0 + SLAB] = res.results[core]["out"].reshape(C, SLAB, W, D)
    return out


if __name__ == "__main__":
    rng = np.random.default_rng(0)
    ins = {
        "x": rng.standard_normal((B, C, H, W, D), np.float32),
        "wq": rng.standard_normal((C, C), np.float32) / 16,
        "bq": rng.standard_normal(C).astype(np.float32) * 0.01,
        "wk": rng.standard_normal((C, C), np.float32) / 16,
        "bk": rng.standard_normal(C).astype(np.float32) * 0.01,
        "wv": rng.standard_normal((C, C), np.float32) / 16,
        "bv": rng.standard_normal(C).astype(np.float32) * 0.01,
        "wp": rng.standard_normal((C, C), np.float32) / 16,
        "bp": rng.standard_normal(C).astype(np.float32) * 0.01,
    }
    out = kernel(**ins)
    print("kernel ran, out shape", out.shape, "mean", float(np.abs(out).mean()))



# revision 48
# speedup vs baseline: 1.0054x; 1.0054x over previous
"""AxialAttention Trainium2 kernel (8 NeuronCores, SPMD).

Sharding: core = b*4 + q  (b in {0,1}, q in {0..3}); each core handles one
batch element and a 10-row H-slab, with all 256 channels. The three
"branches" in the reference are numerically identical (h=w=d=40 and the
reshape ignores the axis names), so out = 3 * branch; the factor 3 is folded
into wp. The attention scale is folded into wq/bq.

Layouts use a "double deck": channel c (<128) lives at partition band 0-39,
channel c+128 at band 64-103 (both legal matmul base partitions). Pivot
transposes place the two decks in one psum tile via tile_position col
offsets, so psum->SBUF evacuations move both decks per op.

The slice loop is software-pipelined for the in-order PE queue:
  phase 1: pivot transposes(i) interleaved with pivot-back+wp(i-1)
  phase 2: attention(i) (scores issued one group ahead of AV) interleaved
           with the qkv conv of slice i+1.
"""

import sys

sys.path.insert(0, "/opt/trn_rl_repo")

import numpy as np
import ml_dtypes
from contextlib import ExitStack

import concourse.bass as bass
import concourse.tile as tile
from concourse import bacc, mybir
from concourse.bass_utils import run_bass_kernel_spmd
from concourse.masks import make_identity

BF16 = mybir.dt.bfloat16
F32 = mybir.dt.float32

B, C, H, W, D = 2, 256, 40, 40, 40
HEADS = 8
HD = C // HEADS
SCALE = HD ** -0.5
N_CORES = 8
SLAB = H // 4          # 10 H-rows per core
WD = W * D             # 1600
NSLAB = SLAB * WD      # 16000
CHALF = 128


def _merge(a, b):
    """Proportionally interleave two chunk lists, preserving each order."""
    out = []
    na, nb = len(a), len(b)
    ia = ib = 0
    while ia < na or ib < nb:
        if ib >= nb or (ia * (nb + 1) <= ib * (na + 1) and ia < na):
            out.append(a[ia])
            ia += 1
        else:
            out.append(b[ib])
            ib += 1
    return out


def _build_nc():
    nc = bacc.Bacc(
        "TRN2",
        target_bir_lowering=False,
        debug=False,
        num_devices=N_CORES,
    )
    x_d = nc.declare_dram_parameter("x", [C, NSLAB], BF16, isOutput=False)
    wqkv_d = nc.declare_dram_parameter("wqkv", [C, 3 * C], BF16, isOutput=False)
    bqkv_d = nc.declare_dram_parameter("bqkv", [3 * C, 1], F32, isOutput=False)
    wp_d = nc.declare_dram_parameter("wp3", [C, C], BF16, isOutput=False)
    bp_d = nc.declare_dram_parameter("bp", [C, 1], F32, isOutput=False)
    out_d = nc.declare_dram_parameter("out", [C, NSLAB], F32, isOutput=True)

    IDENT = mybir.ActivationFunctionType.Identity
    EXP = mybir.ActivationFunctionType.Exp
    MULT = mybir.AluOpType.mult

    with ExitStack() as ctx:
        tc = ctx.enter_context(tile.TileContext(nc))
        const = ctx.enter_context(tc.tile_pool(name="const", bufs=1))
        xp = ctx.enter_context(tc.tile_pool(name="xp", bufs=2))
        qkvp = ctx.enter_context(tc.tile_pool(name="qkvp", bufs=2))
        attp = ctx.enter_context(tc.tile_pool(name="attp", bufs=2))
        ep = ctx.enter_context(tc.tile_pool(name="ep", bufs=3))
        oallp = ctx.enter_context(tc.tile_pool(name="oallp", bufs=2))
        # branch/bt are produced and consumed within one pbwp (a full slice
        # before their next writer): single buffer suffices
        brp = ctx.enter_context(tc.tile_pool(name="brp", bufs=1))
        btp = ctx.enter_context(tc.tile_pool(name="btp", bufs=1))
        recp = ctx.enter_context(tc.tile_pool(name="recp", bufs=4))
        outp = ctx.enter_context(tc.tile_pool(name="outp", bufs=2))
        # conv and wp psums share one 4-deep tag; scores shares ps_t with the
        # pivot transposes (time-disjoint phases); o_ps and pb share ps_x
        ps_cs = ctx.enter_context(tc.tile_pool(name="ps_cs", bufs=4, space="PSUM"))
        ps_t = ctx.enter_context(tc.tile_pool(name="ps_t", bufs=2, space="PSUM"))
        ps_x = ctx.enter_context(tc.tile_pool(name="ps_x", bufs=2, space="PSUM"))

        ident = const.tile([128, 128], BF16)
        make_identity(nc, ident[:])

        # qkv weights DMA goes first: it gates the very first conv matmul.
        # Split by contraction half — the k=0 matmuls only need the first.
        # wp/bp (needed ~12us in) are issued after x(0) — see below.
        wqkv_sb = const.tile([128, 2, 3 * C], BF16)
        wqkv_src = wqkv_d.ap().rearrange("(ko ki) m -> ki ko m", ki=128)
        nc.sync.dma_start(wqkv_sb[:, 0], wqkv_src[:, 0])
        nc.sync.dma_start(wqkv_sb[:, 1], wqkv_src[:, 1])
        bqkv_sb = const.tile([128, 6, 1], F32)
        nc.sync.dma_start(
            bqkv_sb[:], bqkv_d.ap().rearrange("(mo mi) one -> mi mo one", mi=128)
        )
        wp_sb = const.tile([128, 2, C], BF16)
        bp_sb = const.tile([128, 2, 1], F32)

        def load_weights_tail():
            nc.sync.dma_start(
                wp_sb[:], wp_d.ap().rearrange("(ko ki) m -> ki ko m", ki=128)
            )
            nc.sync.dma_start(
                bp_sb[:], bp_d.ap().rearrange("(mo mi) one -> mi mo one", mi=128)
            )

        # channel-pair groups (deck covers c_local and c_local+128)
        groups = []
        c0 = 0
        while c0 < CHALF:
            groups.append((c0, min(12, CHALF - c0)))
            c0 += 12

        def load_x(i, defer=False):
            # split into 4 chunks: the first conv matmul only needs chunk 0,
            # and small chunks interleave with out-store DMAs on the shared
            # DMA engines instead of queuing behind a full-slice transfer.
            # With defer=True the dma_starts are returned as chunks so they
            # can be issued after the pivot-back xbar transposes.
            x_sb = xp.tile([128, 2, WD], BF16, name="x_sb")
            chunks = []
            for n in range(4):
                def ch(n=n):
                    nc.sync.dma_start(
                        x_sb[:, :, n * 400 : (n + 1) * 400],
                        x_d.ap()[
                            :, i * WD + n * 400 : i * WD + (n + 1) * 400
                        ].rearrange("(ko ki) n -> ki ko n", ki=128),
                    )
                chunks.append(ch)
            if not defer:
                for ch in chunks:
                    ch()
                chunks = []
            return x_sb, chunks

        def conv_emit(x_sb):
            # deck-interleaved padded layout (3 tensors q,k,v):
            #   q,k: col = w*128 + deck*64 + d      (t = 0, 1)
            #   v:   col = d*128 + deck*64 + w      (t = 2)
            # so one contiguous 128-col slice holds both decks of one w (or d)
            # and a single PE transpose pivots both. Pad cols 40-63 per deck
            # stay garbage; their transposed rows are never read.
            qkv_sb = qkvp.tile([128, 3, W * 128], BF16, name="qkv_sb")
            qk_view = qkv_sb.rearrange("p t (w g d) -> p t w g d", g=2, d=64)
            v_view = qkv_sb.rearrange("p t (d g w) -> p t d g w", g=2, w=64)
            chunks = []
            for m in range(6):
                for n in range(4):
                    def ch(m=m, n=n, x_sb=x_sb):
                        ps = ps_cs.tile(
                            [128, 512], F32, tag="ps_cs", name="conv_ps"
                        )[:, 0:400]
                        for k in range(2):
                            nc.tensor.matmul(
                                ps[:],
                                lhsT=wqkv_sb[:, k, m * 128 : (m + 1) * 128],
                                rhs=x_sb[:, k, n * 400 : (n + 1) * 400],
                                start=(k == 0),
                                stop=(k == 1),
                            )
                        t, g = m // 2, m % 2
                        if m < 4:
                            # psum cols enumerate (w outer, d inner)
                            dst = qk_view[:, t, n * 10 : (n + 1) * 10, g, 0:40]
                        else:
                            dst = v_view[:, t, 0:40, g, n * 10 : (n + 1) * 10].rearrange(
                                "p d w -> p w d"
                            )
                        # conv evacuation overlaps the attention phase, where
                        # the scalar engine also runs exp: put 1/3 on vector
                        if n == 0 or (n == 1 and m < 2):
                            nc.vector.tensor_scalar_add(dst, ps[:], bqkv_sb[:, m])
                        else:
                            nc.scalar.activation(
                                out=dst,
                                in_=ps[:],
                                func=IDENT,
                                bias=bqkv_sb[:, m],
                                scale=1.0,
                            )
                    chunks.append(ch)
            return qkv_sb, chunks

        def pivots_emit(qkv_sb):
            q_att = attp.tile([128, W * CHALF], BF16, tag="q_att", name="q_att")
            k_att = attp.tile([128, W * CHALF], BF16, tag="k_att", name="k_att")
            v_att = attp.tile([128, 41 * CHALF], BF16, tag="v_att", name="v_att")
            chunks = [
                lambda: nc.vector.memset(v_att[:, 40 * CHALF : 41 * CHALF], 1.0)
            ]
            # one transpose per (tensor, w-or-d): in [128ch, 128 cols] holds
            # both decks of one w (d-padded) -> out [128, 128] with deck0
            # rows 0-39 and deck1 rows 64-103 (pad rows 40-63/104-127 are
            # junk, never read).
            # q/k pivots first so the first scores' dependencies clear while
            # the v pivots (needed one stage later, at the first AV) still run
            # the scalar engine is idle during the pivot phase (exp/conv
            # evacs overlap attention): give it q's evacuations; the vector
            # engine (2x on bf16 psum) takes k and v
            for src, dst, eng in (
                (0, q_att, "scalar"),
                (1, k_att, "vector"),
                (2, v_att, "vector"),
            ):
                for wg in range(10):
                    def ch(wg=wg, src=src, dst=dst, eng=eng):
                        pst = ps_t.tile([128, 512], BF16, tag="pst", name="pst")
                        for wl in range(4):
                            w = wg * 4 + wl
                            nc.tensor.transpose(
                                pst[:, wl * 128 : (wl + 1) * 128],
                                qkv_sb[:, src, w * 128 : (w + 1) * 128],
                                ident[:],
                            )
                        if eng == "scalar":
                            nc.scalar.copy(
                                dst[0:104, wg * 512 : (wg + 1) * 512], pst[0:104, :]
                            )
                        else:
                            nc.vector.tensor_copy(
                                out=dst[0:104, wg * 512 : (wg + 1) * 512],
                                in_=pst[0:104, :],
                            )
                    chunks.append(ch)
            return (q_att, k_att, v_att), chunks

        def attn_emit(att):
            q_att, k_att, v_att = att
            k_v = k_att.rearrange("p (w c) -> p c w", c=CHALF)
            q_v = q_att.rearrange("p (w c) -> p c w", c=CHALF)
            vv = v_att.rearrange("p (d c) -> p c d", c=CHALF)
            o_all = oallp.tile([128, CHALF * W], BF16, name="o_all")

            def scores_stage(c0, gn):
                s_ps = ps_t.tile([128, 512], F32, tag="pst", name="s_ps")
                for j in range(gn):
                    for cc in range(2):
                        r0 = cc * 64
                        nc.tensor.matmul(
                            s_ps[r0 : r0 + 40, j * 40 : (j + 1) * 40],
                            lhsT=k_v[r0 : r0 + 40, c0 + j],
                            rhs=q_v[r0 : r0 + 40, c0 + j],
                            start=True,
                            stop=True,
                        )
                e_sb = ep.tile([128, 480], BF16, tag="e_sb", name="e_sb")
                nc.scalar.activation(
                    out=e_sb[0:104, : gn * 40], in_=s_ps[0:104, : gn * 40], func=EXP
                )
                return e_sb

            def av_stage(c0, gn, e_sb):
                o_ps = ps_x.tile([128, 512], F32, tag="ps_x", name="o_ps")
                for j in range(gn):
                    for cc in range(2):
                        r0 = cc * 64
                        nc.tensor.matmul(
                            o_ps[r0 : r0 + 41, j * 40 : (j + 1) * 40],
                            lhsT=vv[r0 : r0 + 40, c0 + j],
                            rhs=e_sb[r0 : r0 + 40, j * 40 : (j + 1) * 40],
                            start=True,
                            stop=True,
                        )
                nc.vector.tensor_copy(
                    out=o_all[0:105, c0 * 40 : c0 * 40 + gn * 40],
                    in_=o_ps[0:105, : gn * 40],
                )

            pend = [None]
            chunks = []
            for c0, gn in groups:
                def ch(c0=c0, gn=gn):
                    e_sb = scores_stage(c0, gn)
                    if pend[0] is not None:
                        av_stage(*pend[0])
                    pend[0] = (c0, gn, e_sb)
                chunks.append(ch)
            chunks.append(lambda: av_stage(*pend[0]))
            return o_all, chunks

        def pbwp_emit(o_all, i, last=False):
            branch_sb = brp.tile([128, 2, WD], BF16, name="branch_sb")
            out_sb = outp.tile([128, 2, WD], F32, name="out_sb")
            # pivot-back via DMA xbar transpose instead of PE: o_all is
            # [112 rows (d decks + denoms + junk), c*40+w cols] bf16 in SBUF;
            # one transpose per channel-half yields bt[c, w, q] = o_all[q,
            # c*40+w]. The c-split lets half 0 start after the 6th o_all
            # evacuation, mid-attention.
            # returns (early, late): early chunks (pivot-back + normalize)
            # merge into the next pivot phase; late chunks (wp matmuls +
            # stores) append after it, so the in-order PE queue never parks
            # on a wp matmul while pivot transposes are ready to run
            chunks = []
            if not last:
                bt = btp.tile([128, W, 112], BF16, name="bt")
                rec = recp.tile([128, W, 2], F32, tag="rec", name="rec")
                for h in range(2):
                    def ch(h=h):
                        p0, p1 = h * 64, (h + 1) * 64
                        nc.sync.dma_start(
                            bt[p0:p1],
                            o_all[0:112, h * 2560 : (h + 1) * 2560],
                            transpose=True,
                        )
                        nc.vector.reciprocal(rec[p0:p1, :, 0], bt[p0:p1, :, 40])
                        nc.vector.reciprocal(rec[p0:p1, :, 1], bt[p0:p1, :, 104])
                    chunks.append(ch)
                # full-width multiplies, one per wp n-chunk's w-range
                for q in range(4):
                    def ch(q=q):
                        w0, w1 = q * 10, (q + 1) * 10
                        for cc in range(2):
                            nc.vector.tensor_tensor(
                                branch_sb[:, cc].rearrange(
                                    "p (w d) -> p w d", d=40
                                )[:, w0:w1],
                                bt[:, w0:w1, cc * 64 : cc * 64 + 40],
                                rec[:, w0:w1, cc : cc + 1].to_broadcast(
                                    (128, 10, 40)
                                ),
                                MULT,
                            )
                    chunks.append(ch)
            else:
                # final slice: PE pivot-back — its latency chain is shorter
                # than the xbar round-trip, and nothing overlaps the tail
                o_v = o_all.rearrange("p (c w) -> p w c", w=W)
                for wb in range(5):
                    def ch(wb=wb):
                        pb_full = ps_x.tile(
                            [128, 1024], BF16, tag="ps_x", name="pb"
                        )
                        pb = pb_full[:, 0:848]
                        for wl in range(8):
                            w = wb * 8 + wl
                            nc.tensor.transpose(
                                pb[:, wl * 106 : wl * 106 + 105],
                                o_v[0:105, w, :],
                                ident[0:105, 0:105],
                            )
                        pb_v = pb.rearrange("p (w q) -> p w q", q=106)
                        rec = recp.tile([128, 8, 2], F32, tag="rec", name="rec")
                        nc.vector.reciprocal(rec[:, :, 0], pb_v[:, :, 40])
                        nc.vector.reciprocal(rec[:, :, 1], pb_v[:, :, 104])
                        for cc in range(2):
                            nc.vector.tensor_tensor(
                                branch_sb[:, cc].rearrange(
                                    "p (w d) -> p w d", d=40
                                )[:, wb * 8 : wb * 8 + 8],
                                pb_v[:, :, cc * 64 : cc * 64 + 40],
                                rec[:, :, cc : cc + 1].to_broadcast((128, 8, 40)),
                                MULT,
                            )
                    chunks.append(ch)
            early, chunks = chunks, []
            # n-major so each store chunk can fire right after its two wp
            # evacuations; on the last slice both engines are idle, so
            # alternate the evacuations between them to shorten the tail
            for n in range(4):
                for m in range(2):
                    def ch(m=m, n=n):
                        ps = ps_cs.tile(
                            [128, 512], F32, tag="ps_cs", name="wp_ps"
                        )[:, 0:400]
                        for k in range(2):
                            nc.tensor.matmul(
                                ps[:],
                                lhsT=wp_sb[:, k, m * 128 : (m + 1) * 128],
                                rhs=branch_sb[:, k, n * 400 : (n + 1) * 400],
                                start=(k == 0),
                                stop=(k == 1),
                            )
                        if last and m == 1:
                            nc.vector.tensor_scalar_add(
                                out_sb[:, m, n * 400 : (n + 1) * 400],
                                ps[:],
                                bp_sb[:, m],
                            )
                        else:
                            nc.scalar.activation(
                                out=out_sb[:, m, n * 400 : (n + 1) * 400],
                                in_=ps[:],
                                func=IDENT,
                                bias=bp_sb[:, m],
                                scale=1.0,
                            )
                    chunks.append(ch)

                # split store: chunk dn only needs wp evacs (0,dn) and (1,dn)
                def dma_ch(dn=n):
                    nc.scalar.dma_start(
                        out_d.ap()[
                            :, i * WD + dn * 400 : i * WD + (dn + 1) * 400
                        ].rearrange("(ko ki) n -> ki ko n", ki=128),
                        out_sb[:, :, dn * 400 : (dn + 1) * 400],
                    )
                chunks.append(dma_ch)
            return early, chunks

        # ---- software-pipelined slice loop
        x_cur, _ = load_x(0)
        load_weights_tail()
        qkv_cur, conv_ch = conv_emit(x_cur)
        for ch in conv_ch:
            ch()
        pending_pbwp = []
        for i in range(SLAB):
            if i + 1 < SLAB:
                # defer the x dma_starts to after the pivot phase so the
                # pivot-back xbar transposes lead the DMA-engine queue
                x_nxt, x_ch = load_x(i + 1, defer=True)
            else:
                x_ch = []
            att, piv_ch = pivots_emit(qkv_cur)
            # pbwp "early" chunks (xbar pivot-back + normalize) merge into
            # the pivot stream after a short lead (they wait on the previous
            # slice's last o_all evacuation); the wp matmuls and stores go
            # after all pivot transposes so the in-order PE queue never
            # parks on them
            pb_early, pb_late = pending_pbwp if pending_pbwp else ([], [])
            for ch in piv_ch[:2] + _merge(piv_ch[2:], pb_early) + pb_late:
                ch()
            o_all, attn_ch = attn_emit(att)
            if i + 1 < SLAB:
                qkv_nxt, conv_ch = conv_emit(x_nxt)
            else:
                qkv_nxt, conv_ch = None, []
            for ch in x_ch + _merge(attn_ch, conv_ch):
                ch()
            pending_pbwp = pbwp_emit(o_all, i, last=(i == SLAB - 1))
            qkv_cur = qkv_nxt
        for ch in pending_pbwp[0] + pending_pbwp[1]:
            ch()

    nc.compile()
    return nc


_NC_CACHE = None


def _get_nc():
    global _NC_CACHE
    if _NC_CACHE is None:
        _NC_CACHE = _build_nc()
    return _NC_CACHE


def make_in_maps(x, wq, bq, wk, bk, wv, bv, wp, bp):
    bf = ml_dtypes.bfloat16
    wqkv = np.concatenate(
        [wq.T * SCALE, wk.T, wv.T], axis=1
    ).astype(bf)  # [C, 3C], lhsT layout (c_in rows, c_out cols)
    bqkv = np.concatenate([bq * SCALE, bk, bv]).reshape(3 * C, 1).astype(np.float32)
    wp3 = (3.0 * wp).T.astype(bf)  # [C, C]
    bp_ = bp.reshape(C, 1).astype(np.float32)
    in_maps = []
    for core in range(N_CORES):
        b = core // 4
        r0 = (core % 4) * SLAB
        x_slab = np.ascontiguousarray(
            x[b, :, r0 : r0 + SLAB].reshape(C, NSLAB)
        ).astype(bf)
        in_maps.append(
            {"x": x_slab, "wqkv": wqkv, "bqkv": bqkv, "wp3": wp3, "bp": bp_}
        )
    return in_maps


def run_on_cores(in_maps, **kw):
    nc = _get_nc()
    return run_bass_kernel_spmd(nc, in_maps, core_ids=list(range(N_CORES)), **kw)


def kernel(x, wq, bq, wk, bk, wv, bv, wp, bp):
    x = np.asarray(x, dtype=np.float32)
    in_maps = make_in_maps(
        x,
        np.asarray(wq, np.float32),
        np.asarray(bq, np.float32),
        np.asarray(wk, np.float32),
        np.asarray(bk, np.float32),
        np.asarray(wv, np.float32),
        np.asarray(bv, np.float32),
        np.asarray(wp, np.float32),
        np.asarray(bp, np.float32),
    )
    res = run_on_cores(in_maps)
    out = np.empty((B, C, H, W, D), np.float32)
    for core in range(N_CORES):
        b = core // 4
        r0 = (core % 4) * SLAB
        out[b, :, r0 : r0 + SLAB] = res.results[core]["out"].reshape(C, SLAB, W, D)
    return out


if __name__ == "__main__":
    rng = np.random.default_rng(0)
    ins = {
        "x": rng.standard_normal((B, C, H, W, D), np.float32),
        "wq": rng.standard_normal((C, C), np.float32) / 16,
        "bq": rng.standard_normal(C).astype(np.float32) * 0.01,
        "wk": rng.standard_normal((C, C), np.float32) / 16,
        "bk": rng.standard_normal(C).astype(np.float32) * 0.01,
        "wv": rng.standard_normal((C, C), np.float32) / 16,
        "bv": rng.standard_normal(C).astype(np.float32) * 0.01,
        "wp": rng.standard_normal((C, C), np.float32) / 16,
        "bp": rng.standard_normal(C).astype(np.float32) * 0.01,
    }
    out = kernel(**ins)
    print("kernel ran, out shape", out.shape, "mean", float(np.abs(out).mean()))



# revision 51
# speedup vs baseline: 1.0118x; 1.0064x over previous
"""AxialAttention Trainium2 kernel (8 NeuronCores, SPMD).

Sharding: core = b*4 + q  (b in {0,1}, q in {0..3}); each core handles one
batch element and a 10-row H-slab, with all 256 channels. The three
"branches" in the reference are numerically identical (h=w=d=40 and the
reshape ignores the axis names), so out = 3 * branch; the factor 3 is folded
into wp. The attention scale is folded into wq/bq.

Layouts use a "double deck": channel c (<128) lives at partition band 0-39,
channel c+128 at band 64-103 (both legal matmul base partitions). Pivot
transposes place the two decks in one psum tile via tile_position col
offsets, so psum->SBUF evacuations move both decks per op.

The slice loop is software-pipelined for the in-order PE queue:
  phase 1: pivot transposes(i) interleaved with pivot-back+wp(i-1)
  phase 2: attention(i) (scores issued one group ahead of AV) interleaved
           with the qkv conv of slice i+1.
"""

import sys

sys.path.insert(0, "/opt/trn_rl_repo")

import numpy as np
import ml_dtypes
from contextlib import ExitStack

import concourse.bass as bass
import concourse.tile as tile
from concourse import bacc, mybir
from concourse.bass_utils import run_bass_kernel_spmd
from concourse.masks import make_identity

BF16 = mybir.dt.bfloat16
F32 = mybir.dt.float32

B, C, H, W, D = 2, 256, 40, 40, 40
HEADS = 8
HD = C // HEADS
SCALE = HD ** -0.5
N_CORES = 8
SLAB = H // 4          # 10 H-rows per core
WD = W * D             # 1600
NSLAB = SLAB * WD      # 16000
CHALF = 128


def _merge(a, b):
    """Proportionally interleave two chunk lists, preserving each order."""
    out = []
    na, nb = len(a), len(b)
    ia = ib = 0
    while ia < na or ib < nb:
        if ib >= nb or (ia * (nb + 1) <= ib * (na + 1) and ia < na):
            out.append(a[ia])
            ia += 1
        else:
            out.append(b[ib])
            ib += 1
    return out


def _build_nc():
    nc = bacc.Bacc(
        "TRN2",
        target_bir_lowering=False,
        debug=False,
        num_devices=N_CORES,
    )
    x_d = nc.declare_dram_parameter("x", [C, NSLAB], BF16, isOutput=False)
    wqkv_d = nc.declare_dram_parameter("wqkv", [C, 3 * C], BF16, isOutput=False)
    bqkv_d = nc.declare_dram_parameter("bqkv", [3 * C, 1], F32, isOutput=False)
    wp_d = nc.declare_dram_parameter("wp3", [C, C], BF16, isOutput=False)
    bp_d = nc.declare_dram_parameter("bp", [C, 1], F32, isOutput=False)
    out_d = nc.declare_dram_parameter("out", [C, NSLAB], F32, isOutput=True)

    IDENT = mybir.ActivationFunctionType.Identity
    EXP = mybir.ActivationFunctionType.Exp
    MULT = mybir.AluOpType.mult

    with ExitStack() as ctx:
        tc = ctx.enter_context(tile.TileContext(nc))
        const = ctx.enter_context(tc.tile_pool(name="const", bufs=1))
        xp = ctx.enter_context(tc.tile_pool(name="xp", bufs=2))
        qkvp = ctx.enter_context(tc.tile_pool(name="qkvp", bufs=2))
        attp = ctx.enter_context(tc.tile_pool(name="attp", bufs=2))
        ep = ctx.enter_context(tc.tile_pool(name="ep", bufs=3))
        oallp = ctx.enter_context(tc.tile_pool(name="oallp", bufs=2))
        # branch/bt are produced and consumed within one pbwp (a full slice
        # before their next writer): single buffer suffices
        brp = ctx.enter_context(tc.tile_pool(name="brp", bufs=1))
        btp = ctx.enter_context(tc.tile_pool(name="btp", bufs=1))
        recp = ctx.enter_context(tc.tile_pool(name="recp", bufs=4))
        outp = ctx.enter_context(tc.tile_pool(name="outp", bufs=2))
        # conv and wp psums share one 4-deep tag; scores shares ps_t with the
        # pivot transposes (time-disjoint phases); o_ps and pb share ps_x
        ps_cs = ctx.enter_context(tc.tile_pool(name="ps_cs", bufs=4, space="PSUM"))
        ps_t = ctx.enter_context(tc.tile_pool(name="ps_t", bufs=2, space="PSUM"))
        ps_x = ctx.enter_context(tc.tile_pool(name="ps_x", bufs=2, space="PSUM"))

        ident = const.tile([128, 128], BF16)
        make_identity(nc, ident[:])

        # qkv weights DMA goes first: it gates the very first conv matmul.
        # Split by contraction half — the k=0 matmuls only need the first.
        # wp/bp (needed ~12us in) are issued after x(0) — see below.
        wqkv_sb = const.tile([128, 2, 3 * C], BF16)
        wqkv_src = wqkv_d.ap().rearrange("(ko ki) m -> ki ko m", ki=128)
        nc.sync.dma_start(wqkv_sb[:, 0], wqkv_src[:, 0])
        nc.sync.dma_start(wqkv_sb[:, 1], wqkv_src[:, 1])
        bqkv_sb = const.tile([128, 6, 1], F32)
        nc.sync.dma_start(
            bqkv_sb[:], bqkv_d.ap().rearrange("(mo mi) one -> mi mo one", mi=128)
        )
        wp_sb = const.tile([128, 2, C], BF16)
        bp_sb = const.tile([128, 2, 1], F32)

        def load_weights_tail():
            nc.sync.dma_start(
                wp_sb[:], wp_d.ap().rearrange("(ko ki) m -> ki ko m", ki=128)
            )
            nc.sync.dma_start(
                bp_sb[:], bp_d.ap().rearrange("(mo mi) one -> mi mo one", mi=128)
            )

        # channel-pair groups (deck covers c_local and c_local+128)
        groups = []
        c0 = 0
        while c0 < CHALF:
            groups.append((c0, min(12, CHALF - c0)))
            c0 += 12

        def load_x(i, defer=False):
            # split into 4 chunks: the first conv matmul only needs chunk 0,
            # and small chunks interleave with out-store DMAs on the shared
            # DMA engines instead of queuing behind a full-slice transfer.
            # With defer=True the dma_starts are returned as chunks so they
            # can be issued after the pivot-back xbar transposes.
            x_sb = xp.tile([128, 2, WD], BF16, name="x_sb")
            chunks = []
            for n in range(4):
                def ch(n=n):
                    nc.sync.dma_start(
                        x_sb[:, :, n * 400 : (n + 1) * 400],
                        x_d.ap()[
                            :, i * WD + n * 400 : i * WD + (n + 1) * 400
                        ].rearrange("(ko ki) n -> ki ko n", ki=128),
                    )
                chunks.append(ch)
            if not defer:
                for ch in chunks:
                    ch()
                chunks = []
            return x_sb, chunks

        def conv_emit(x_sb):
            # deck-interleaved padded layout (3 tensors q,k,v):
            #   q,k: col = w*128 + deck*64 + d      (t = 0, 1)
            #   v:   col = d*128 + deck*64 + w      (t = 2)
            # so one contiguous 128-col slice holds both decks of one w (or d)
            # and a single PE transpose pivots both. Pad cols 40-63 per deck
            # stay garbage; their transposed rows are never read.
            qkv_sb = qkvp.tile([128, 3, W * 128], BF16, name="qkv_sb")
            qk_view = qkv_sb.rearrange("p t (w g d) -> p t w g d", g=2, d=64)
            v_view = qkv_sb.rearrange("p t (d g w) -> p t d g w", g=2, w=64)
            chunks = []
            for m in range(6):
                for n in range(4):
                    def ch(m=m, n=n, x_sb=x_sb):
                        ps = ps_cs.tile(
                            [128, 512], F32, tag="ps_cs", name="conv_ps"
                        )[:, 0:400]
                        for k in range(2):
                            nc.tensor.matmul(
                                ps[:],
                                lhsT=wqkv_sb[:, k, m * 128 : (m + 1) * 128],
                                rhs=x_sb[:, k, n * 400 : (n + 1) * 400],
                                start=(k == 0),
                                stop=(k == 1),
                            )
                        t, g = m // 2, m % 2
                        if m < 4:
                            # psum cols enumerate (w outer, d inner)
                            dst = qk_view[:, t, n * 10 : (n + 1) * 10, g, 0:40]
                        else:
                            dst = v_view[:, t, 0:40, g, n * 10 : (n + 1) * 10].rearrange(
                                "p d w -> p w d"
                            )
                        # a few late-chunk evacuations on vector to balance
                        # the scalar engine (which also runs exp)
                        if m >= 2 and n == 0:
                            nc.vector.tensor_scalar_add(dst, ps[:], bqkv_sb[:, m])
                        else:
                            nc.scalar.activation(
                                out=dst,
                                in_=ps[:],
                                func=IDENT,
                                bias=bqkv_sb[:, m],
                                scale=1.0,
                            )
                    chunks.append(ch)
            return qkv_sb, chunks

        def pivots_emit(qkv_sb):
            q_att = attp.tile([128, W * CHALF], BF16, tag="q_att", name="q_att")
            k_att = attp.tile([128, W * CHALF], BF16, tag="k_att", name="k_att")
            v_att = attp.tile([128, 41 * CHALF], BF16, tag="v_att", name="v_att")
            chunks = [
                lambda: nc.vector.memset(v_att[:, 40 * CHALF : 41 * CHALF], 1.0)
            ]
            # one transpose per (tensor, w-or-d): in [128ch, 128 cols] holds
            # both decks of one w (d-padded) -> out [128, 128] with deck0
            # rows 0-39 and deck1 rows 64-103 (pad rows 40-63/104-127 are
            # junk, never read).
            # q/k pivots first so the first scores' dependencies clear while
            # the v pivots (needed one stage later, at the first AV) still run
            # the scalar engine is idle during the pivot phase (exp/conv
            # evacs overlap attention): give it q's evacuations; the vector
            # engine (2x on bf16 psum) takes k and v
            for src, dst, eng in (
                (0, q_att, "scalar"),
                (1, k_att, "vector"),
                (2, v_att, "vector"),
            ):
                for wg in range(10):
                    def ch(wg=wg, src=src, dst=dst, eng=eng):
                        pst = ps_t.tile([128, 512], BF16, tag="pst", name="pst")
                        for wl in range(4):
                            w = wg * 4 + wl
                            nc.tensor.transpose(
                                pst[:, wl * 128 : (wl + 1) * 128],
                                qkv_sb[:, src, w * 128 : (w + 1) * 128],
                                ident[:],
                            )
                        if eng == "scalar":
                            nc.scalar.copy(
                                dst[0:104, wg * 512 : (wg + 1) * 512], pst[0:104, :]
                            )
                        else:
                            nc.vector.tensor_copy(
                                out=dst[0:104, wg * 512 : (wg + 1) * 512],
                                in_=pst[0:104, :],
                            )
                    chunks.append(ch)
            return (q_att, k_att, v_att), chunks

        def attn_emit(att):
            q_att, k_att, v_att = att
            k_v = k_att.rearrange("p (w c) -> p c w", c=CHALF)
            q_v = q_att.rearrange("p (w c) -> p c w", c=CHALF)
            vv = v_att.rearrange("p (d c) -> p c d", c=CHALF)
            o_all = oallp.tile([128, CHALF * W], BF16, name="o_all")

            def scores_stage(c0, gn):
                s_ps = ps_t.tile([128, 512], F32, tag="pst", name="s_ps")
                for j in range(gn):
                    for cc in range(2):
                        r0 = cc * 64
                        nc.tensor.matmul(
                            s_ps[r0 : r0 + 40, j * 40 : (j + 1) * 40],
                            lhsT=k_v[r0 : r0 + 40, c0 + j],
                            rhs=q_v[r0 : r0 + 40, c0 + j],
                            start=True,
                            stop=True,
                        )
                e_sb = ep.tile([128, 480], BF16, tag="e_sb", name="e_sb")
                nc.scalar.activation(
                    out=e_sb[0:104, : gn * 40], in_=s_ps[0:104, : gn * 40], func=EXP
                )
                return e_sb

            def av_stage(c0, gn, e_sb):
                o_ps = ps_x.tile([128, 512], F32, tag="ps_x", name="o_ps")
                for j in range(gn):
                    for cc in range(2):
                        r0 = cc * 64
                        nc.tensor.matmul(
                            o_ps[r0 : r0 + 41, j * 40 : (j + 1) * 40],
                            lhsT=vv[r0 : r0 + 40, c0 + j],
                            rhs=e_sb[r0 : r0 + 40, j * 40 : (j + 1) * 40],
                            start=True,
                            stop=True,
                        )
                nc.vector.tensor_copy(
                    out=o_all[0:105, c0 * 40 : c0 * 40 + gn * 40],
                    in_=o_ps[0:105, : gn * 40],
                )

            pend = [None]
            chunks = []
            for c0, gn in groups:
                def ch(c0=c0, gn=gn):
                    e_sb = scores_stage(c0, gn)
                    if pend[0] is not None:
                        av_stage(*pend[0])
                    pend[0] = (c0, gn, e_sb)
                chunks.append(ch)
            chunks.append(lambda: av_stage(*pend[0]))
            return o_all, chunks

        def pbwp_emit(o_all, i, last=False):
            branch_sb = brp.tile([128, 2, WD], BF16, name="branch_sb")
            out_sb = outp.tile([128, 2, WD], F32, name="out_sb")
            # pivot-back via DMA xbar transpose instead of PE: o_all is
            # [112 rows (d decks + denoms + junk), c*40+w cols] bf16 in SBUF;
            # one transpose per channel-half yields bt[c, w, q] = o_all[q,
            # c*40+w]. The c-split lets half 0 start after the 6th o_all
            # evacuation, mid-attention.
            # returns (early, late): early chunks (pivot-back + normalize)
            # merge into the next pivot phase; late chunks (wp matmuls +
            # stores) append after it, so the in-order PE queue never parks
            # on a wp matmul while pivot transposes are ready to run
            chunks = []
            if not last:
                bt = btp.tile([128, W, 112], BF16, name="bt")
                rec = recp.tile([128, W, 2], F32, tag="rec", name="rec")

                def dmat_ch():
                    # SP-only dispatch: keeps the (slow-to-resolve) xbar
                    # transposes out of the vector engine's in-order queue
                    for h in range(2):
                        nc.sync.dma_start(
                            bt[h * 64 : (h + 1) * 64],
                            o_all[0:112, h * 2560 : (h + 1) * 2560],
                            transpose=True,
                        )
                chunks.append(dmat_ch)
                # recips + full-width multiplies, one per wp n-chunk's
                # w-range; scheduled late in the merge so they don't park
                # the vector queue while the xbar transposes run
                for q in range(4):
                    def ch(q=q):
                        if q == 0:
                            nc.vector.reciprocal(rec[:, :, 0], bt[:, :, 40])
                            nc.vector.reciprocal(rec[:, :, 1], bt[:, :, 104])
                        w0, w1 = q * 10, (q + 1) * 10
                        for cc in range(2):
                            nc.vector.tensor_tensor(
                                branch_sb[:, cc].rearrange(
                                    "p (w d) -> p w d", d=40
                                )[:, w0:w1],
                                bt[:, w0:w1, cc * 64 : cc * 64 + 40],
                                rec[:, w0:w1, cc : cc + 1].to_broadcast(
                                    (128, 10, 40)
                                ),
                                MULT,
                            )
                    chunks.append(ch)
            else:
                # final slice: PE pivot-back — its latency chain is shorter
                # than the xbar round-trip, and nothing overlaps the tail
                o_v = o_all.rearrange("p (c w) -> p w c", w=W)
                for wb in range(5):
                    def ch(wb=wb):
                        pb_full = ps_x.tile(
                            [128, 1024], BF16, tag="ps_x", name="pb"
                        )
                        pb = pb_full[:, 0:848]
                        for wl in range(8):
                            w = wb * 8 + wl
                            nc.tensor.transpose(
                                pb[:, wl * 106 : wl * 106 + 105],
                                o_v[0:105, w, :],
                                ident[0:105, 0:105],
                            )
                        pb_v = pb.rearrange("p (w q) -> p w q", q=106)
                        rec = recp.tile([128, 8, 2], F32, tag="rec", name="rec")
                        nc.vector.reciprocal(rec[:, :, 0], pb_v[:, :, 40])
                        nc.vector.reciprocal(rec[:, :, 1], pb_v[:, :, 104])
                        for cc in range(2):
                            nc.vector.tensor_tensor(
                                branch_sb[:, cc].rearrange(
                                    "p (w d) -> p w d", d=40
                                )[:, wb * 8 : wb * 8 + 8],
                                pb_v[:, :, cc * 64 : cc * 64 + 40],
                                rec[:, :, cc : cc + 1].to_broadcast((128, 8, 40)),
                                MULT,
                            )
                    chunks.append(ch)
            early, chunks = chunks, []
            # n-major so each store chunk can fire right after its two wp
            # evacuations; on the last slice both engines are idle, so
            # alternate the evacuations between them to shorten the tail
            for n in range(4):
                for m in range(2):
                    def ch(m=m, n=n):
                        ps = ps_cs.tile(
                            [128, 512], F32, tag="ps_cs", name="wp_ps"
                        )[:, 0:400]
                        for k in range(2):
                            nc.tensor.matmul(
                                ps[:],
                                lhsT=wp_sb[:, k, m * 128 : (m + 1) * 128],
                                rhs=branch_sb[:, k, n * 400 : (n + 1) * 400],
                                start=(k == 0),
                                stop=(k == 1),
                            )
                        if last and m == 1:
                            nc.vector.tensor_scalar_add(
                                out_sb[:, m, n * 400 : (n + 1) * 400],
                                ps[:],
                                bp_sb[:, m],
                            )
                        else:
                            nc.scalar.activation(
                                out=out_sb[:, m, n * 400 : (n + 1) * 400],
                                in_=ps[:],
                                func=IDENT,
                                bias=bp_sb[:, m],
                                scale=1.0,
                            )
                    chunks.append(ch)

                # split store: chunk dn only needs wp evacs (0,dn) and (1,dn)
                def dma_ch(dn=n):
                    nc.scalar.dma_start(
                        out_d.ap()[
                            :, i * WD + dn * 400 : i * WD + (dn + 1) * 400
                        ].rearrange("(ko ki) n -> ki ko n", ki=128),
                        out_sb[:, :, dn * 400 : (dn + 1) * 400],
                    )
                chunks.append(dma_ch)
            return early, chunks

        # ---- software-pipelined slice loop
        # Phase A of slice i runs: pivots(i), pbwp(i-1) (xbar pivot-back
        # early, wp matmuls late), and the first conv chunks of slice i+1
        # (balances PE across phases). Phase B runs: attention(i), the rest
        # of conv(i+1), and dispatches x(i+2) (so the x DMAs never queue
        # ahead of the xbar transposes).
        x_cur, _ = load_x(0)
        load_weights_tail()
        qkv_cur, conv_ch = conv_emit(x_cur)
        for ch in conv_ch:
            ch()
        x_nxt, x_ch_nxt = load_x(1, defer=True)
        for ch in x_ch_nxt:
            ch()
        pending_pbwp = ([], [])
        for i in range(SLAB):
            if i + 1 < SLAB:
                qkv_nxt, conv_ch = conv_emit(x_nxt)
            else:
                qkv_nxt, conv_ch = None, []
            conv_pre, conv_rest = conv_ch[:7], conv_ch[7:]
            att, piv_ch = pivots_emit(qkv_cur)
            pb_early, pb_late = pending_pbwp
            dmat, pb_mid = pb_early[:1], pb_early[1:]
            for ch in (
                dmat
                + piv_ch[:2]
                + _merge(piv_ch[2:], conv_pre + pb_mid)
                + pb_late
            ):
                ch()
            o_all, attn_ch = attn_emit(att)
            if i + 2 < SLAB:
                x_nxt, x_ch = load_x(i + 2, defer=True)
            else:
                x_nxt, x_ch = None, []
            for ch in x_ch + _merge(attn_ch, conv_rest):
                ch()
            pending_pbwp = pbwp_emit(o_all, i, last=(i == SLAB - 1))
            qkv_cur = qkv_nxt
        for ch in pending_pbwp[0] + pending_pbwp[1]:
            ch()

    nc.compile()
    return nc


_NC_CACHE = None


def _get_nc():
    global _NC_CACHE
    if _NC_CACHE is None:
        _NC_CACHE = _build_nc()
    return _NC_CACHE


def make_in_maps(x, wq, bq, wk, bk, wv, bv, wp, bp):
    bf = ml_dtypes.bfloat16
    wqkv = np.concatenate(
        [wq.T * SCALE, wk.T, wv.T], axis=1
    ).astype(bf)  # [C, 3C], lhsT layout (c_in rows, c_out cols)
    bqkv = np.concatenate([bq * SCALE, bk, bv]).reshape(3 * C, 1).astype(np.float32)
    wp3 = (3.0 * wp).T.astype(bf)  # [C, C]
    bp_ = bp.reshape(C, 1).astype(np.float32)
    in_maps = []
    for core in range(N_CORES):
        b = core // 4
        r0 = (core % 4) * SLAB
        x_slab = np.ascontiguousarray(
            x[b, :, r0 : r0 + SLAB].reshape(C, NSLAB)
        ).astype(bf)
        in_maps.append(
            {"x": x_slab, "wqkv": wqkv, "bqkv": bqkv, "wp3": wp3, "bp": bp_}
        )
    return in_maps


def run_on_cores(in_maps, **kw):
    nc = _get_nc()
    return run_bass_kernel_spmd(nc, in_maps, core_ids=list(range(N_CORES)), **kw)


def kernel(x, wq, bq, wk, bk, wv, bv, wp, bp):
    x = np.asarray(x, dtype=np.float32)
    in_maps = make_in_maps(
        x,
        np.asarray(wq, np.float32),
        np.asarray(bq, np.float32),
        np.asarray(wk, np.float32),
        np.asarray(bk, np.float32),
        np.asarray(wv, np.float32),
        np.asarray(bv, np.float32),
        np.asarray(wp, np.float32),
        np.asarray(bp, np.float32),
    )
    res = run_on_cores(in_maps)
    out = np.empty((B, C, H, W, D), np.float32)
    for core in range(N_CORES):
        b = core // 4
        r0 = (core % 4) * SLAB
        out[b, :, r0 : r0 + SLAB] = res.results[core]["out"].reshape(C, SLAB, W, D)
    return out


if __name__ == "__main__":
    rng = np.random.default_rng(0)
    ins = {
        "x": rng.standard_normal((B, C, H, W, D), np.float32),
        "wq": rng.standard_normal((C, C), np.float32) / 16,
        "bq": rng.standard_normal(C).astype(np.float32) * 0.01,
        "wk": rng.standard_normal((C, C), np.float32) / 16,
        "bk": rng.standard_normal(C).astype(np.float32) * 0.01,
        "wv": rng.standard_normal((C, C), np.float32) / 16,
        "bv": rng.standard_normal(C).astype(np.float32) * 0.01,
        "wp": rng.standard_normal((C, C), np.float32) / 16,
        "bp": rng.standard_normal(C).astype(np.float32) * 0.01,
    }
    out = kernel(**ins)
    print("kernel ran, out shape", out.shape, "mean", float(np.abs(out).mean()))

